# revision 1
# baseline (speedup 1.0000x reference)
"""Int8 AG-GEMM (x @ weight.T with per-row/per-col dequant + bias) on 8 TRN2
NeuronCores.

Strategy: data-parallel over M (rows of x). Core c owns rows
[c*512, (c+1)*512). All inputs are fed fully prepared from the host:
  - xt   [K, M_C]  bf16 : transposed x shard (int8 values, exact in bf16)
  - wt   [K, N]    bf16 : transposed weight (replicated to every core)
  - isr  [128, M_C] f32 : input_scale shard replicated across partitions
  - wsr  [N/128, 128] f32 : weight_scale tiled per n-tile
  - br   [N/128, 128] f32 : bias tiled per n-tile
Each core computes outT = [N, M_C] bf16 (the transposed output shard):
  psum[n-tile 128, M_C] = sum_k wt_tile[k, n].T @ xt_tile[k]   (fp32, exact)
  out = (psum * isr) * ws[n][:,1] + bias[n][:,1]  -> bf16
The host transposes each core's outT back and stitches the full [M, N].

The int8 GEMM is exact: int8 values are exact in bf16, products are exact in
the PE's fp32 accumulator, and partial sums stay far below 2^24.
"""

import numpy as np

M_FULL, K_FULL, N_FULL = 4096, 8192, 8192
N_CORES = 8


def build_nc(K, N, M_C, n_per_blk=256):
    """Build the SPMD kernel graph for per-core problem [K, N] x [K, M_C]."""
    import concourse.bass as bass  # noqa: F401
    import concourse.mybir as mybir
    import concourse.tile as tile
    from concourse import bacc

    bf16 = mybir.dt.bfloat16
    f32 = mybir.dt.float32

    kt = K // 128          # k-tiles
    nt = N // 128          # n-tiles (output partition tiles)
    nblk = N // n_per_blk  # weight streaming super-blocks
    jt = n_per_blk // 128  # n-tiles per super-block

    nc = bacc.Bacc("TRN2", target_bir_lowering=False, debug=False,
                   num_devices=N_CORES)
    xt = nc.dram_tensor("xt", [K, M_C], bf16, kind="ExternalInput")
    wt = nc.dram_tensor("wt", [K, N], bf16, kind="ExternalInput")
    isr = nc.dram_tensor("isr", [128, M_C], f32, kind="ExternalInput")
    wsr = nc.dram_tensor("wsr", [nt, 128], f32, kind="ExternalInput")
    br = nc.dram_tensor("br", [nt, 128], f32, kind="ExternalInput")
    outt = nc.dram_tensor("outt", [N, M_C], bf16, kind="ExternalOutput")

    with tile.TileContext(nc) as tc:
        with (
            tc.tile_pool(name="const", bufs=1) as cpool,
            tc.tile_pool(name="wstream", bufs=3) as wpool,
            tc.tile_pool(name="psum", bufs=4, space="PSUM") as ppool,
            tc.tile_pool(name="t1", bufs=4) as t1pool,
            tc.tile_pool(name="osb", bufs=4) as opool,
        ):
            xsb = cpool.tile([128, kt, M_C], bf16)
            nc.sync.dma_start(xsb[:], xt.ap().rearrange("(k p) m -> p k m", p=128))
            isr_sb = cpool.tile([128, M_C], f32)
            nc.sync.dma_start(isr_sb[:], isr.ap())
            ws_sb = cpool.tile([128, nt], f32)
            nc.sync.dma_start(ws_sb[:], wsr.ap().rearrange("n p -> p n"))
            b_sb = cpool.tile([128, nt], f32)
            nc.sync.dma_start(b_sb[:], br.ap().rearrange("n p -> p n"))

            for s in range(nblk):
                wsb = wpool.tile([128, kt, n_per_blk], bf16)
                nc.sync.dma_start(
                    wsb[:],
                    wt.ap()[:, s * n_per_blk:(s + 1) * n_per_blk]
                    .rearrange("(k p) j -> p k j", p=128),
                )
                for j in range(jt):
                    n = s * jt + j
                    ps = ppool.tile([128, M_C], f32)
                    for k in range(kt):
                        nc.tensor.matmul(
                            ps[:],
                            wsb[:, k, j * 128:(j + 1) * 128],
                            xsb[:, k, :],
                            start=(k == 0),
                            stop=(k == kt - 1),
                        )
                    t1 = t1pool.tile([128, M_C], f32)
                    nc.vector.tensor_tensor(
                        t1[:], ps[:], isr_sb[:], mybir.AluOpType.mult
                    )
                    ob = opool.tile([128, M_C], bf16)
                    nc.scalar.activation(
                        ob[:], t1[:],
                        mybir.ActivationFunctionType.Identity,
                        bias=b_sb[:, n:n + 1],
                        scale=ws_sb[:, n:n + 1],
                    )
                    nc.sync.dma_start(outt.ap()[n * 128:(n + 1) * 128, :], ob[:])

    nc.compile()
    return nc


def prep_in_maps(x, weight, bias, input_scale, weight_scale, n_cores=N_CORES):
    """Host-side shard + layout prep. Returns (in_maps, M_C)."""
    import ml_dtypes

    bf16 = ml_dtypes.bfloat16
    M, K = x.shape
    N = weight.shape[0]
    M_C = M // n_cores

    xt_full = np.ascontiguousarray(x.T).astype(np.float32).astype(bf16)  # [K, M]
    wt = np.ascontiguousarray(weight.T).astype(np.float32).astype(bf16)  # [K, N]
    wsr = np.ascontiguousarray(weight_scale.astype(np.float32).reshape(N // 128, 128))
    br = np.ascontiguousarray(bias.astype(np.float32).reshape(N // 128, 128))

    in_maps = []
    for c in range(n_cores):
        sl = slice(c * M_C, (c + 1) * M_C)
        in_maps.append({
            "xt": np.ascontiguousarray(xt_full[:, sl]),
            "wt": wt,
            "isr": np.ascontiguousarray(
                np.broadcast_to(input_scale[sl].astype(np.float32)[None, :],
                                (128, M_C))),
            "wsr": wsr,
            "br": br,
        })
    return in_maps, M_C


def run(x, weight, bias, input_scale, weight_scale, trace=False):
    """Run the SPMD kernel; returns (out [M, N] bf16, BassKernelResults)."""
    from concourse.bass_utils import run_bass_kernel_spmd

    M, K = x.shape
    N = weight.shape[0]
    in_maps, M_C = prep_in_maps(x, weight, bias, input_scale, weight_scale)
    nc = build_nc(K, N, M_C)
    res = run_bass_kernel_spmd(nc, in_maps, list(range(N_CORES)), trace=trace)

    out = np.empty((M, N), dtype=in_maps[0]["xt"].dtype)  # bf16
    for c in range(N_CORES):
        out[c * M_C:(c + 1) * M_C, :] = res.results[c]["outt"].T
    return out, res


def kernel(x, weight, bias, input_scale, weight_scale):
    out, _ = run(x, weight, bias, input_scale, weight_scale, trace=False)
    return out


# revision 2
# speedup vs baseline: 1.0300x; 1.0300x over previous
"""Int8 AG-GEMM (x @ weight.T with per-row/per-col dequant + bias) on 8 TRN2
NeuronCores.

Strategy: data-parallel over M (rows of x). Core c owns rows
[c*512, (c+1)*512). All inputs are fed fully prepared from the host:
  - xt   [K, M_C]  bf16 : transposed x shard (int8 values, exact in bf16)
  - wt   [K, N]    bf16 : transposed weight (replicated to every core)
  - isr  [128, M_C] f32 : input_scale shard replicated across partitions
  - wsr  [N/128, 128] f32 : weight_scale tiled per n-tile
  - br   [N/128, 128] f32 : bias tiled per n-tile
Each core computes outT = [N, M_C] bf16 (the transposed output shard):
  psum[n-tile 128, M_C] = sum_k wt_tile[k, n].T @ xt_tile[k]   (fp32, exact)
  out = (psum * isr) * ws[n][:,1] + bias[n][:,1]  -> bf16
The host transposes each core's outT back and stitches the full [M, N].

The int8 GEMM is exact: int8 values are exact in bf16, products are exact in
the PE's fp32 accumulator, and partial sums stay far below 2^24.
"""

import numpy as np

M_FULL, K_FULL, N_FULL = 4096, 8192, 8192
N_CORES = 8


def build_nc(K, N, M_C, n_per_blk=256):
    """Build the SPMD kernel graph for per-core problem [K, N] x [K, M_C]."""
    import concourse.bass as bass  # noqa: F401
    import concourse.mybir as mybir
    import concourse.tile as tile
    from concourse import bacc

    bf16 = mybir.dt.bfloat16
    f32 = mybir.dt.float32

    kt = K // 128          # k-tiles
    nt = N // 128          # n-tiles (output partition tiles)
    nblk = N // n_per_blk  # weight streaming super-blocks
    jt = n_per_blk // 128  # n-tiles per super-block

    nc = bacc.Bacc("TRN2", target_bir_lowering=False, debug=False,
                   num_devices=N_CORES)
    xt = nc.dram_tensor("xt", [K, M_C], bf16, kind="ExternalInput")
    wt = nc.dram_tensor("wt", [K, N], bf16, kind="ExternalInput")
    isr = nc.dram_tensor("isr", [128, M_C], f32, kind="ExternalInput")
    wsr = nc.dram_tensor("wsr", [nt, 128], f32, kind="ExternalInput")
    br = nc.dram_tensor("br", [nt, 128], f32, kind="ExternalInput")
    outt = nc.dram_tensor("outt", [N, M_C], bf16, kind="ExternalOutput")

    # x chunks: split the resident-x load so the first matmuls can start
    # as soon as the first chunk + first weight half-block land. The x
    # chunks ride the ACT HWDGE queue; the weight stream rides SP, so the
    # two proceed concurrently instead of serializing on one sequencer.
    n_xchunk = 4
    kc = kt // n_xchunk  # k-tiles per x chunk
    kh = kt // 2         # k-tiles per weight half-block

    with tile.TileContext(nc) as tc:
        with (
            tc.tile_pool(name="const", bufs=1) as cpool,
            tc.tile_pool(name="wstream", bufs=3) as wpool,
            tc.tile_pool(name="psum", bufs=4, space="PSUM") as ppool,
            tc.tile_pool(name="t1", bufs=4) as t1pool,
            tc.tile_pool(name="osb", bufs=4) as opool,
        ):
            xt_r = xt.ap().rearrange("(c k p) m -> c p k m", p=128, k=kc)
            xch = []
            for c in range(n_xchunk):
                xc = cpool.tile([128, kc, M_C], bf16, tag=f"xsb{c}")
                nc.scalar.dma_start(xc[:], xt_r[c])
                xch.append(xc)
            isr_sb = cpool.tile([128, M_C], f32)
            nc.scalar.dma_start(isr_sb[:], isr.ap())
            ws_sb = cpool.tile([128, nt], f32)
            nc.scalar.dma_start(ws_sb[:], wsr.ap().rearrange("n p -> p n"))
            b_sb = cpool.tile([128, nt], f32)
            nc.scalar.dma_start(b_sb[:], br.ap().rearrange("n p -> p n"))

            for s in range(nblk):
                wt_s = (wt.ap()[:, s * n_per_blk:(s + 1) * n_per_blk]
                        .rearrange("(h k p) j -> h p k j", p=128, k=kh))
                wlo = wpool.tile([128, kh, n_per_blk], bf16, tag="wlo")
                nc.sync.dma_start(wlo[:], wt_s[0])
                whi = wpool.tile([128, kh, n_per_blk], bf16, tag="whi")
                nc.sync.dma_start(whi[:], wt_s[1])
                for j in range(jt):
                    n = s * jt + j
                    ps = ppool.tile([128, M_C], f32)
                    for k in range(kt):
                        wk = wlo if k < kh else whi
                        nc.tensor.matmul(
                            ps[:],
                            wk[:, k % kh, j * 128:(j + 1) * 128],
                            xch[k // kc][:, k % kc, :],
                            start=(k == 0),
                            stop=(k == kt - 1),
                        )
                    t1 = t1pool.tile([128, M_C], f32)
                    nc.vector.tensor_tensor(
                        t1[:], ps[:], isr_sb[:], mybir.AluOpType.mult
                    )
                    ob = opool.tile([128, M_C], bf16)
                    nc.scalar.activation(
                        ob[:], t1[:],
                        mybir.ActivationFunctionType.Identity,
                        bias=b_sb[:, n:n + 1],
                        scale=ws_sb[:, n:n + 1],
                    )
                    nc.sync.dma_start(outt.ap()[n * 128:(n + 1) * 128, :], ob[:])

    nc.compile()
    return nc


def prep_in_maps(x, weight, bias, input_scale, weight_scale, n_cores=N_CORES):
    """Host-side shard + layout prep. Returns (in_maps, M_C)."""
    import ml_dtypes

    bf16 = ml_dtypes.bfloat16
    M, K = x.shape
    N = weight.shape[0]
    M_C = M // n_cores

    xt_full = np.ascontiguousarray(x.T).astype(np.float32).astype(bf16)  # [K, M]
    wt = np.ascontiguousarray(weight.T).astype(np.float32).astype(bf16)  # [K, N]
    wsr = np.ascontiguousarray(weight_scale.astype(np.float32).reshape(N // 128, 128))
    br = np.ascontiguousarray(bias.astype(np.float32).reshape(N // 128, 128))

    in_maps = []
    for c in range(n_cores):
        sl = slice(c * M_C, (c + 1) * M_C)
        in_maps.append({
            "xt": np.ascontiguousarray(xt_full[:, sl]),
            "wt": wt,
            "isr": np.ascontiguousarray(
                np.broadcast_to(input_scale[sl].astype(np.float32)[None, :],
                                (128, M_C))),
            "wsr": wsr,
            "br": br,
        })
    return in_maps, M_C


def run(x, weight, bias, input_scale, weight_scale, trace=False):
    """Run the SPMD kernel; returns (out [M, N] bf16, BassKernelResults)."""
    from concourse.bass_utils import run_bass_kernel_spmd

    M, K = x.shape
    N = weight.shape[0]
    in_maps, M_C = prep_in_maps(x, weight, bias, input_scale, weight_scale)
    nc = build_nc(K, N, M_C)
    res = run_bass_kernel_spmd(nc, in_maps, list(range(N_CORES)), trace=trace)

    out = np.empty((M, N), dtype=in_maps[0]["xt"].dtype)  # bf16
    for c in range(N_CORES):
        out[c * M_C:(c + 1) * M_C, :] = res.results[c]["outt"].T
    return out, res


def kernel(x, weight, bias, input_scale, weight_scale):
    out, _ = run(x, weight, bias, input_scale, weight_scale, trace=False)
    return out
